# revision 31
# baseline (speedup 1.0000x reference)
"""Self-contained Trainium2 Bass kernel for GQA attention (RoPE + causal).

Problem: hidden (B=2, S=2048, HID=2048), W_qkv (3072, 2048) = 16 Q heads +
2*4 KV heads of dim 128, RoPE, causal GQA attention, W_o (2048, 2048).

Sharding: 8 cores = (batch b in {0,1}) x (KV group g in {0..3}).  Each core
gets 4 Q heads + 1 KV head (the GQA group stays intact), computes its
partial output through the 512 matching W_o columns, and the host sums the
4 partials per batch (the unshard step).  No on-device collectives.

Layout strategy: everything is kept "transposed" (feature dim on SBUF
partitions) so every matmul contraction lands on the partition axis with no
device-side transposes of big tensors:
  - host passes hidden[b].T, W_shard.T, W_o_shard.T, cos.T, signed sin.T
  - QKV projection emits q^T/k^T/v^T directly (d on partitions)
  - RoPE's rotate_half runs on the vector engine as two partition-crossed
    half-tile multiplies against a sign-folded sin table (no PE matmul)
  - scores are computed transposed: S^T[t, sq] = (k^T tile).T @ q^T, so the
    softmax sum over t is a ones-vector matmul and exp(S^T) feeds the PV
    matmul directly (no P-tile transposes); only V needs 16 tiny PE
    transposes to its natural (t, d) layout.
  - attention output appears as out^T (d on partitions) which is exactly
    the stationary operand the W_o projection wants.
Matmul operands are bf16 (1 cycle/row on TensorE); accumulation/softmax
arithmetic stays f32.

Perf notes (from perfetto traces): the kernel is TensorE-bound (~94% PE
occupancy), so the wins here are (a) warm-up matmuls on a zeroed tile while
the first DMAs land, beating the PE HAM clock-gate ramp, (b) RoPE rotate
moved off the PE, (c) causal-mask multiply on VectorE instead of GpSimd
(short exp->PV critical path), (d) Wo PSUM banks evacuated by Scalar+Vector
halves in parallel and stored as one contiguous 512KB row DMA per seq tile.
"""

import math

import numpy as np
import ml_dtypes

import concourse.bass as bass
import concourse.bacc as bacc
import concourse.mybir as mybir
from concourse.tile import TileContext
from concourse.bass_utils import run_bass_kernel_spmd

F32 = mybir.dt.float32
BF16 = mybir.dt.bfloat16
AF = mybir.ActivationFunctionType

P = 128  # SBUF partitions / head dim / tile edge


def build_attn_nc(S=2048, HID=2048, NQ=4, HD=128, SC=512):
    """One NeuronCore graph: NQ query heads + 1 KV head, full sequence."""
    assert HD == P
    n_h = HID // P   # contraction tiles of the QKV projection
    n_st = S // P    # 128-row tiles of the sequence
    n_sc = S // SC   # 512-wide chunks of the sequence
    n_tc = SC // P   # t-tiles per sq chunk (diagonal mask patterns)
    NO = NQ + 2      # projection output tiles: q0..q{NQ-1}, k, v
    n_ec = HID // SC
    H2 = HD // 2
    scale = 1.0 / math.sqrt(HD)

    nc = bacc.Bacc("TRN2", target_bir_lowering=False, debug=False, num_devices=8)
    hidT = nc.declare_dram_parameter("hidT", [HID, S], BF16, isOutput=False)
    wqkvT = nc.declare_dram_parameter("wqkvT", [HID, NO * P], BF16, isOutput=False)
    woT = nc.declare_dram_parameter("woT", [NQ * HD, HID], BF16, isOutput=False)
    cosT = nc.declare_dram_parameter("cosT", [HD, S], BF16, isOutput=False)
    sinnT = nc.declare_dram_parameter("sinnT", [HD, S], BF16, isOutput=False)
    ident = nc.declare_dram_parameter("ident", [P, P], BF16, isOutput=False)
    maskT = nc.declare_dram_parameter("maskT", [P, P], BF16, isOutput=False)
    out = nc.declare_dram_parameter("out", [S, HID], BF16, isOutput=True)

    with TileContext(nc) as tc:
        with (
            tc.tile_pool(name="const", bufs=1) as constp,
            tc.tile_pool(name="wbf", bufs=1) as wbfp,
            tc.tile_pool(name="big", bufs=n_h) as bigp,
            tc.tile_pool(name="raw", bufs=4) as rawp,
            tc.tile_pool(name="act", bufs=1) as actp,
            tc.tile_pool(name="tmp", bufs=2) as tmpp,
            tc.tile_pool(name="rrp", bufs=3) as rrp,
            tc.tile_pool(name="es", bufs=16) as esp,
            tc.tile_pool(name="outsb", bufs=3) as outp,
            tc.tile_pool(name="acc", bufs=2, space="PSUM") as accp,
            tc.tile_pool(name="wop", bufs=2, space="PSUM") as wop,
            tc.tile_pool(name="scp", bufs=3, space="PSUM") as scp,
            tc.tile_pool(name="rsp", bufs=1, space="PSUM") as rsp,
        ):
            # ---------------- constants + PE warm-up ----------------------
            warm_sb = constp.tile([P, SC], BF16, name="warm_sb")
            nc.vector.memset(warm_sb[:, :], 0.0)
            ones_sq = constp.tile([P, P], BF16, name="ones_sq")
            nc.gpsimd.memset(ones_sq[:, :], 1.0)

            # warm the PE's HAM clock gate while the first weight/hidden
            # DMAs are in flight: a few cold 512-wide matmuls on a zero tile
            for i in range(5):
                wps = rsp.tile([P, SC], F32, tag="rs", name=f"warm{i}")
                nc.tensor.matmul(wps[:, :], lhsT=warm_sb[:, :P],
                                 rhs=warm_sb[:, :], start=True, stop=True)

            # ---------------- load weights / hidden (bf16 direct) ---------
            # kv weight columns + hidden tiles first so the k/v projection
            # can start as soon as the first pair lands; rope/mask constants
            # next (needed ~40us in); q columns last (needed by pass 2)
            w_bf, hid_bf = [], []
            for h in range(n_h):
                wb = wbfp.tile([P, NO * P], BF16, tag=f"wbf{h}", name=f"wb{h}")
                nc.sync.dma_start(out=wb[:, NQ * P:],
                                  in_=wqkvT[h * P:(h + 1) * P, NQ * P:])
                w_bf.append(wb)
                hb = bigp.tile([P, S], BF16, tag="big", name=f"hb{h}")
                nc.sync.dma_start(out=hb[:, :], in_=hidT[h * P:(h + 1) * P, :])
                hid_bf.append(hb)
            # first few q columns before the rope tables (pass 2 starts on
            # them right as pass 1 ends), remaining q columns after
            for h in range(4):
                nc.sync.dma_start(out=w_bf[h][:, :NQ * P],
                                  in_=wqkvT[h * P:(h + 1) * P, :NQ * P])
            cos_sb = constp.tile([HD, S], BF16, name="cos_sb")
            nc.sync.dma_start(out=cos_sb[:, :], in_=cosT[:, :])
            sinn_sb = constp.tile([HD, S], BF16, name="sinn_sb")
            nc.sync.dma_start(out=sinn_sb[:, :], in_=sinnT[:, :])
            for h in range(4, n_h):
                nc.sync.dma_start(out=w_bf[h][:, :NQ * P],
                                  in_=wqkvT[h * P:(h + 1) * P, :NQ * P])
            id_sb = constp.tile([P, P], BF16, name="id_sb")
            nc.sync.dma_start(out=id_sb[:, :], in_=ident[:, :])
            mask_sb = constp.tile([P, P], BF16, name="mask_sb")
            nc.sync.dma_start(out=mask_sb[:, :], in_=maskT[:, :])

            qhat = [actp.tile([HD, S], BF16, tag=f"qhat{i}", name=f"qhat{i}")
                    for i in range(NQ)]
            khat = actp.tile([HD, S], BF16, tag="khat", name="khat")
            V_bf = actp.tile([P, S], BF16, tag="V", name="V_bf")

            # ---------------- QKV projection (+RoPE / V transpose) --------
            def rope_of(o, rawo):
                # dest = raw*cos + rotate_half(raw)*sin, all on VectorE.
                # rotate_half is two half multiplies whose OUTPUT partition
                # base is shifted by 64; sinn is the half-swapped sign-folded
                # sin table so both DVE inputs share a base partition.
                # Chunked big-chunk-first so attention on chunk 3 can start
                # before the whole tensor is roped.
                dest = qhat[o] if o < NQ else khat
                t1 = tmpp.tile([P, S], BF16, tag="t1", name=f"t1_{o}")
                t2 = tmpp.tile([P, S], BF16, tag="t2", name=f"t2_{o}")
                for sc in reversed(range(n_sc)):
                    c = slice(sc * SC, (sc + 1) * SC)
                    nc.vector.tensor_mul(t1[0:H2, c], rawo[H2:HD, c],
                                         sinn_sb[H2:HD, c])
                    nc.vector.tensor_mul(t1[H2:HD, c], rawo[0:H2, c],
                                         sinn_sb[0:H2, c])
                    nc.vector.tensor_mul(t2[:, c], rawo[:, c], cos_sb[:, c])
                    nc.vector.tensor_add(dest[:, c], t1[:, c], t2[:, c])

            def vtrans_of(rawo):
                for st in range(n_st):
                    pst = scp.tile([P, P], BF16, tag="sc", name=f"psv{st}")
                    nc.tensor.transpose(pst[:, :], rawo[:, st * P:(st + 1) * P],
                                        id_sb[:, :])
                    nc.scalar.copy(out=V_bf[:, st * P:(st + 1) * P], in_=pst[:, :])

            # first pass: k and v together, h-outer, using all 8 psum banks —
            # doubles the PE work available per arriving hidden tile while
            # the input stream is the bottleneck
            ps_k = [accp.tile([P, SC], F32, tag="acc", name=f"pspk{sc}")
                    for sc in range(2)] + \
                   [wop.tile([P, SC], F32, tag="wo", name=f"pspk{sc}")
                    for sc in range(2, n_sc)]
            ps_v = [scp.tile([P, SC], F32, tag="sc", name=f"pspv{sc}")
                    for sc in range(min(2, n_sc))] + \
                   [rsp.tile([P, SC], F32, tag="rs", name="pspv2")
                    for _ in range(1 if n_sc > 2 else 0)] + \
                   [scp.tile([P, SC], F32, tag="sc", name="pspv3")
                    for _ in range(1 if n_sc > 3 else 0)]
            for h in range(n_h):
                for sc in range(n_sc):
                    nc.tensor.matmul(
                        ps_k[sc][:, :],
                        lhsT=w_bf[h][:, NQ * P:(NQ + 1) * P],
                        rhs=hid_bf[h][:, sc * SC:(sc + 1) * SC],
                        start=(h == 0), stop=(h == n_h - 1))
                    nc.tensor.matmul(
                        ps_v[sc][:, :],
                        lhsT=w_bf[h][:, (NQ + 1) * P:(NQ + 2) * P],
                        rhs=hid_bf[h][:, sc * SC:(sc + 1) * SC],
                        start=(h == 0), stop=(h == n_h - 1))
            rawk = rawp.tile([P, S], BF16, tag="raw", name="rawk")
            rawv = rawp.tile([P, S], BF16, tag="raw", name="rawv")
            # k banks evacuated first (both engines): pass 2 reuses them
            for sc in range(n_sc):
                nc.scalar.copy(out=rawk[:, sc * SC:sc * SC + SC // 2],
                               in_=ps_k[sc][:, :SC // 2])
                nc.vector.tensor_copy(rawk[:, sc * SC + SC // 2:(sc + 1) * SC],
                                      ps_k[sc][:, SC // 2:])
            for sc in range(n_sc):
                nc.scalar.copy(out=rawv[:, sc * SC:sc * SC + SC // 2],
                               in_=ps_v[sc][:, :SC // 2])
                nc.vector.tensor_copy(rawv[:, sc * SC + SC // 2:(sc + 1) * SC],
                                      ps_v[sc][:, SC // 2:])
            rope_of(NQ, rawk)
            vtrans_of(rawv)

            # remaining passes: q heads, hidden now resident
            for o in range(NQ):
                ps = [(accp.tile([P, SC], F32, tag="acc", name=f"psp{o}_{sc}")
                       if sc < 2 else
                       wop.tile([P, SC], F32, tag="wo", name=f"psp{o}_{sc}"))
                      for sc in range(n_sc)]
                for h in range(n_h):
                    # h=0 forward (hits the first-freed psum banks first);
                    # later h reversed so chunk 3 stops first for the copies
                    sc_iter = (range(n_sc) if h == 0
                               else reversed(range(n_sc)))
                    for sc in sc_iter:
                        nc.tensor.matmul(
                            ps[sc][:, :],
                            lhsT=w_bf[h][:, o * P:(o + 1) * P],
                            rhs=hid_bf[h][:, sc * SC:(sc + 1) * SC],
                            start=(h == 0), stop=(h == n_h - 1))
                rawo = rawp.tile([P, S], BF16, tag="raw", name=f"raw{o}")
                # both engines evacuate in parallel: rope starts sooner.
                # last head frees the accp banks first — attention's first
                # ps_o allocation WARs on them
                order = (range(n_sc) if o == NQ - 1
                         else reversed(range(n_sc)))
                for sc in order:
                    nc.scalar.copy(out=rawo[:, sc * SC:sc * SC + SC // 2],
                                   in_=ps[sc][:, :SC // 2])
                    nc.vector.tensor_copy(rawo[:, sc * SC + SC // 2:(sc + 1) * SC],
                                          ps[sc][:, SC // 2:])
                rope_of(o, rawo)

            ohat = [bigp.tile([HD, S], BF16, tag="big", name=f"ohat{i}")
                    for i in range(NQ)]

            # ---------------- causal attention (transposed form) ----------
            # big chunks first so the final Wo/output tail is short
            for sc in reversed(range(n_sc)):
                csl = slice(sc * SC, (sc + 1) * SC)
                n_t = n_tc * (sc + 1)
                for q in range(NQ):
                    ps_o = accp.tile([HD, SC], F32, tag="acc", name=f"pso{q}_{sc}")
                    ps_r = rsp.tile([P, SC], F32, tag="rs", name=f"psn{q}_{sc}")
                    # software-pipelined: ones/PV trail scores/exp by one
                    # tile, so the first ones-matmul never waits on the
                    # previous chunk's reciprocal reading ps_r, and each
                    # tile's exp gets a full extra tile-time of slack
                    def ones_pv(tt, c0, es):
                        nc.tensor.matmul(ps_r[:, c0:], lhsT=ones_sq[:, :],
                                         rhs=es[:, c0:],
                                         start=(tt == 0), stop=(tt == n_t - 1))
                        nc.tensor.matmul(ps_o[:, c0:],
                                         lhsT=V_bf[:, tt * P:(tt + 1) * P],
                                         rhs=es[:, c0:],
                                         start=(tt == 0), stop=(tt == n_t - 1))

                    pend = None
                    for tt in range(n_t):
                        j = tt - n_tc * sc
                        # diagonal tiles only contribute to sq >= t: trim the
                        # dead columns; the surviving leading 128-block gets
                        # the shared triangle mask
                        c0 = j * P if j > 0 else 0
                        ps_s = scp.tile([P, SC], F32, tag="sc",
                                        name=f"pss{q}_{sc}_{tt}")
                        nc.tensor.matmul(ps_s[:, c0:],
                                         lhsT=khat[:, tt * P:(tt + 1) * P],
                                         rhs=qhat[q][:, sc * SC + c0:(sc + 1) * SC],
                                         start=True, stop=True)
                        es = esp.tile([P, SC], BF16, tag="es",
                                      name=f"es{q}_{sc}_{tt}")
                        nc.scalar.activation(es[:, c0:], ps_s[:, c0:], AF.Exp,
                                             scale=scale)
                        if j >= 0:
                            nc.vector.tensor_mul(es[:, c0:c0 + P],
                                                 es[:, c0:c0 + P],
                                                 mask_sb[:, :])
                        if pend is not None:
                            ones_pv(*pend)
                        pend = (tt, c0, es)
                    ones_pv(*pend)
                    rr = rrp.tile([P, SC], F32, tag="rr", name=f"rr{q}_{sc}")
                    nc.vector.reciprocal_approx_fast(out=rr[:, :], in_=ps_r[:, :])
                    nc.vector.tensor_mul(ohat[q][:, csl], ps_o[:, :], rr[:, :])

            # ---------------- output projection ---------------------------
            wo_bf = []
            for hh in range(NQ):
                wob = bigp.tile([P, HID], BF16, tag="big", name=f"wob{hh}")
                nc.sync.dma_start(out=wob[:, :], in_=woT[hh * P:(hh + 1) * P, :])
                wo_bf.append(wob)
            ec_groups = [list(range(i, min(i + 2, n_ec)))
                         for i in range(0, n_ec, 2)]
            st_order = [st for sc in reversed(range(n_sc))
                        for st in range(sc * n_tc, (sc + 1) * n_tc)]
            grp = 0
            for st in st_order:
                orow = outp.tile([P, HID], BF16, tag="orow", name=f"orow{st}")
                for ecs in ec_groups:
                    # alternate psum pools by group parity: 2+3 banks in
                    # rotation, so a group's first matmul never waits on the
                    # previous group's copy-out
                    pool = wop if grp % 2 == 0 else scp
                    tg = "wo" if grp % 2 == 0 else "sc"
                    grp += 1
                    po = [pool.tile([P, SC], F32, tag=tg, name=f"pw{st}_{ec}")
                          for ec in ecs]
                    for hh in range(NQ):
                        for i, ec in enumerate(ecs):
                            nc.tensor.matmul(
                                po[i][:, :],
                                lhsT=ohat[hh][:, st * P:(st + 1) * P],
                                rhs=wo_bf[hh][:, ec * SC:(ec + 1) * SC],
                                start=(hh == 0), stop=(hh == NQ - 1))
                    # evacuate each psum bank with both engines in parallel
                    # so the bank frees in half the time for the next group
                    for i, ec in enumerate(ecs):
                        nc.scalar.copy(out=orow[:, ec * SC:ec * SC + SC // 2],
                                       in_=po[i][:, :SC // 2])
                        nc.vector.tensor_copy(orow[:, ec * SC + SC // 2:(ec + 1) * SC],
                                              po[i][:, SC // 2:])
                    # store each half-row as soon as its groups' copies land
                    lo = ecs[0] * SC
                    hi = (ecs[-1] + 1) * SC
                    nc.sync.dma_start(out=out[st * P:(st + 1) * P, lo:hi],
                                      in_=orow[:, lo:hi])
    nc.compile()
    return nc


def make_host_constants(S, HD=128, SC=512):
    ident = np.eye(P, dtype=np.float32)
    tt_idx = np.arange(P)[:, None]
    ss_idx = np.arange(P)[None, :]
    mask = (ss_idx >= tt_idx).astype(np.float32)
    bf = ml_dtypes.bfloat16
    return ident.astype(bf), mask.astype(bf)


def make_in_maps(hidden_states, cos, sin, W_qkv, W_o, NH=16, NKV=4, HD=128):
    """Shard the full inputs into 8 per-core input maps."""
    B = hidden_states.shape[0]
    S = hidden_states.shape[1]
    n_rep = NH // NKV
    ident, mask = make_host_constants(S, HD)
    bf = ml_dtypes.bfloat16
    cosT = np.ascontiguousarray(cos.T).astype(bf)
    # half-swapped sign-folded sin for the DVE rotate_half:
    #   rows [0, 64)  = +sin rows [64, 128)   (multiplies raw[0:64])
    #   rows [64,128) = -sin rows [0, 64)     (multiplies raw[64:128])
    sinT = np.ascontiguousarray(sin.T).astype(np.float32)
    sinnT = np.concatenate([sinT[HD // 2:], -sinT[:HD // 2]],
                           axis=0).astype(bf)
    in_maps = []
    for b in range(B):
        hidT = np.ascontiguousarray(hidden_states[b].T).astype(bf)
        for g in range(NKV):
            wq = W_qkv[g * n_rep * HD:(g + 1) * n_rep * HD]
            wk = W_qkv[NH * HD + g * HD: NH * HD + (g + 1) * HD]
            wv = W_qkv[(NH + NKV) * HD + g * HD: (NH + NKV) * HD + (g + 1) * HD]
            wsh = np.concatenate([wq, wk, wv], axis=0)
            wqkvT = np.ascontiguousarray(wsh.T).astype(bf)
            woT = np.ascontiguousarray(
                W_o[:, g * n_rep * HD:(g + 1) * n_rep * HD].T).astype(bf)
            in_maps.append({
                "hidT": hidT, "wqkvT": wqkvT, "woT": woT,
                "cosT": cosT, "sinnT": sinnT,
                "ident": ident, "maskT": mask,
            })
    return in_maps


_NC_CACHE = {}


def kernel(hidden_states, cos, sin, W_qkv, W_o):
    hidden_states = np.asarray(hidden_states, dtype=np.float32)
    cos = np.asarray(cos, dtype=np.float32)
    sin = np.asarray(sin, dtype=np.float32)
    W_qkv = np.asarray(W_qkv, dtype=np.float32)
    W_o = np.asarray(W_o, dtype=np.float32)

    B, S, HID = hidden_states.shape
    HD = cos.shape[-1]
    NH = W_o.shape[1] // HD
    NKV = (W_qkv.shape[0] // HD - NH) // 2
    n_rep = NH // NKV

    key = (S, HID, n_rep, HD)
    if key not in _NC_CACHE:
        _NC_CACHE[key] = build_attn_nc(S=S, HID=HID, NQ=n_rep, HD=HD)
    nc = _NC_CACHE[key]

    in_maps = make_in_maps(hidden_states, cos, sin, W_qkv, W_o, NH, NKV, HD)
    res = run_bass_kernel_spmd(nc, in_maps, core_ids=list(range(B * NKV)))
    outs = [np.asarray(r["out"], dtype=np.float32) for r in res.results]
    full = np.stack(
        [np.sum(outs[b * NKV:(b + 1) * NKV], axis=0, dtype=np.float32)
         for b in range(B)], axis=0)
    return full.astype(np.float32)
